# revision 1
# baseline (speedup 1.0000x reference)
"""Distributed Trainium2 Bass kernel for AtnConv (contextual-attention conv).

Sharding: 8 cores = batch(2) x position-blocks(4). Each core owns 1024 of the
4096 output positions of one sample and attends over ALL L=4096 patches, so the
softmax over L is core-local (free-axis reduction, no collectives needed).

Device per core (bf16 matmuls, fp32 PSUM/softmax):
  scores S^T[pos,L] = X_shard^T @ A'   (A' = normalized patches * mask * SCALE)
  masked softmax over L (stable: block max + deferred rescale)
  U^T[pos, C*4*4]  = Y^T @ R'          (R' = raw 4x4 patches * mask)
Host: im2col prep, col2im scatter-add, clip correction, final 4 dilated convs.
"""

import numpy as np
import ml_dtypes

B, C, H1, H2 = 2, 128, 128, 64
L = H2 * H2            # 4096 patches
POSL = 1024            # positions per core
KDIM = 1152            # 128*3*3 contraction for scores
RDIM = 2048            # 128*4*4 conv-transpose channels
SCALE = 10.0
EPS_NORM = 1e-4
EPS_CLAMP = 1e-8
BF16 = ml_dtypes.bfloat16

_NC = None


def _build_nc():
    import concourse.bass as bass
    import concourse.bacc as bacc
    import concourse.mybir as mybir
    from concourse import tile

    bf = mybir.dt.bfloat16
    f32 = mybir.dt.float32
    Exp = mybir.ActivationFunctionType.Exp
    X = mybir.AxisListType.X

    nc = bacc.Bacc(None, target_bir_lowering=False)
    xT = nc.declare_dram_parameter("xT", [9, 128, POSL], bf, isOutput=False)
    aT = nc.declare_dram_parameter("aT", [9, 128, L], bf, isOutput=False)
    rT = nc.declare_dram_parameter("rT", [32, 128, RDIM], bf, isOutput=False)
    uT = nc.declare_dram_parameter("uT", [POSL, RDIM], f32, isOutput=True)

    with tile.TileContext(nc) as tc:
        with (
            tc.tile_pool(name="big", bufs=1) as big,
            tc.tile_pool(name="st", bufs=1) as st,
            tc.tile_pool(name="wk", bufs=2) as wk,
            tc.tile_pool(name="rp", bufs=1) as rp,
            tc.tile_pool(name="ps", bufs=2, space=bass.MemorySpace.PSUM) as ps,
        ):
            estore = big.tile([128, 8, L], bf)        # 64 KiB/part
            ybuf = big.tile([128, 32, 512], bf)       # 32 KiB/part
            xt = big.tile([128, 9, POSL], bf)         # 18 KiB/part
            nbmaxs = st.tile([128, 8, 8], f32)
            rss = st.tile([128, 8, 8], f32)

            for k in range(9):
                nc.sync.dma_start(xt[:, k, :], xT[k])

            # ---- score matmul + block-local exp ----
            for n in range(8):                        # L blocks of 512
                a_n = wk.tile([128, 9, 512], bf, tag="a_n")
                for k in range(9):
                    nc.sync.dma_start(a_n[:, k, :], aT[k][:, n * 512:(n + 1) * 512])
                for m in range(8):                    # pos tiles of 128
                    z = ps.tile([128, 512], f32, tag="z")
                    for k in range(9):
                        nc.tensor.matmul(
                            z[:], xt[:, k, m * 128:(m + 1) * 128], a_n[:, k, :],
                            start=(k == 0), stop=(k == 8))
                    # nbmaxs holds NEGATED block maxes (reduce negate=True)
                    nc.vector.reduce_max(nbmaxs[:, m, n:n + 1], z[:], axis=X,
                                         negate=True)
                    ef = wk.tile([128, 512], f32, tag="ef")
                    nc.scalar.activation(ef[:], z[:], Exp,
                                         bias=nbmaxs[:, m, n:n + 1], scale=1.0)
                    nc.vector.reduce_sum(rss[:, m, n:n + 1], ef[:], axis=X)
                    nc.vector.tensor_copy(estore[:, m, n * 512:(n + 1) * 512], ef[:])

            # ---- softmax finalize: rescale each block by exp(bmax-gmax)/Z ----
            for m in range(8):
                # ngm = min_n(-bmax_n) = -gmax
                ngm = wk.tile([128, 1], f32, tag="ngm")
                nc.vector.tensor_reduce(ngm[:], nbmaxs[:, m, :], axis=X,
                                        op=mybir.AluOpType.min)
                # al_n = exp(-(nbmax_n)*1 ... ) = exp(bmax_n - gmax)
                al = wk.tile([128, 8], f32, tag="al")
                nc.scalar.activation(al[:], nbmaxs[:, m, :], Exp, bias=ngm[:],
                                     scale=-1.0)
                pr = wk.tile([128, 8], f32, tag="pr")
                nc.vector.tensor_mul(pr[:], al[:], rss[:, m, :])
                sm = wk.tile([128, 1], f32, tag="sm")
                nc.vector.reduce_sum(sm[:], pr[:], axis=X)
                rc = wk.tile([128, 1], f32, tag="rc")
                nc.vector.reciprocal(rc[:], sm[:])
                be = wk.tile([128, 8], f32, tag="be")
                nc.vector.tensor_scalar_mul(be[:], al[:], rc[:])
                for n in range(8):
                    nc.vector.tensor_scalar_mul(
                        estore[:, m, n * 512:(n + 1) * 512],
                        estore[:, m, n * 512:(n + 1) * 512], be[:, n:n + 1])

            # ---- transpose Y^T -> Y, then U^T = Y^T @ R' ----
            for q in range(2):                        # pos halves of 512
                for mm in range(4):
                    for k in range(32):
                        nc.sync.dma_start_transpose(
                            ybuf[:, k, mm * 128:(mm + 1) * 128],
                            estore[:, q * 4 + mm, k * 128:(k + 1) * 128])
                for h in range(4):                    # channel quarters of 512
                    rh = rp.tile([128, 32, 512], bf, tag="rh")
                    for k in range(32):
                        nc.sync.dma_start(rh[:, k, :], rT[k][:, h * 512:(h + 1) * 512])
                    for mm in range(4):
                        u = ps.tile([128, 512], f32, tag="u")
                        for k in range(32):
                            nc.tensor.matmul(
                                u[:], ybuf[:, k, mm * 128:(mm + 1) * 128], rh[:, k, :],
                                start=(k == 0), stop=(k == 31))
                        o = wk.tile([128, 512], f32, tag="o")
                        nc.scalar.copy(o[:], u[:])
                        r0 = q * 512 + mm * 128
                        nc.sync.dma_start(uT[r0:r0 + 128, h * 512:(h + 1) * 512], o[:])
    nc.compile()
    return nc


def _get_nc():
    global _NC
    if _NC is None:
        _NC = _build_nc()
    return _NC


def _im2col3(x, pad):
    # x [C,H,W] -> [C*9, H*W] rows ordered (c,u,v), dilation=pad rate
    Cc, H, W = x.shape
    r = pad
    xp = np.pad(x, ((0, 0), (r, r), (r, r)))
    cols = np.empty((Cc, 3, 3, H, W), dtype=x.dtype)
    for u in range(3):
        for v in range(3):
            cols[:, u, v] = xp[:, u * r:u * r + H, v * r:v * r + W]
    return cols.reshape(Cc * 9, H * W)


def _raw_patches(x1s):
    # x1s [C,128,128] -> R [L, C*16] rows l=(i,j) row-major, cols (c,di,dj)
    xp = np.pad(x1s, ((0, 0), (1, 1), (1, 1)))
    R = np.empty((C, 4, 4, H2, H2), dtype=x1s.dtype)
    for di in range(4):
        for dj in range(4):
            R[:, di, dj] = xp[:, di:di + 128:2, dj:dj + 128:2]
    return R.transpose(3, 4, 0, 1, 2).reshape(L, C * 16)


def _col2im(Ut):
    # Ut [L, C*16] -> y [C,128,128] scatter-add, h=2i+di-1
    blk = Ut.reshape(H2, H2, C, 4, 4).transpose(2, 3, 4, 0, 1)
    acc = np.zeros((C, 130, 130), dtype=np.float32)
    for di in range(4):
        for dj in range(4):
            acc[:, di:di + 128:2, dj:dj + 128:2] += blk[:, di, dj]
    return (acc[:, 1:129, 1:129] / 4.0).astype(np.float32)


def kernel(x1, x2, mask, fw0, fb0, fw1, fb1, fw2, fb2, fw3, fb3):
    from concourse.bass_utils import run_bass_kernel_spmd

    x1 = np.asarray(x1, np.float32)
    x2 = np.asarray(x2, np.float32)
    mask = np.asarray(mask, np.float32)
    fws = [np.asarray(f, np.float32) for f in (fw0, fw1, fw2, fw3)]
    fbs = [np.asarray(f, np.float32) for f in (fb0, fb1, fb2, fb3)]

    nc = _get_nc()
    in_maps = []
    rawRs, mms = [], []
    for s in range(B):
        cols = _im2col3(x2[s], 1)                      # [1152, 4096]
        norms = np.sqrt((cols * cols).sum(0, dtype=np.float32))
        mp = np.pad(mask[s, 0], 1)
        psum = np.zeros((H2, H2), np.float32)
        for u in range(3):
            for v in range(3):
                psum += mp[u:u + H2, v:v + H2]
        mm = (psum.reshape(-1) == 0.0).astype(np.float32)   # [L] 1=valid
        aT = (cols * (SCALE * mm / np.maximum(norms, EPS_NORM))[None, :])
        aT = aT.astype(BF16).reshape(9, 128, L)
        rawR = _raw_patches(x1[s])                     # [L, 2048]
        rT = (rawR * mm[:, None]).astype(BF16).reshape(32, 128, RDIM)
        rawRs.append(rawR)
        mms.append(mm)
        for q in range(4):
            xTq = cols[:, q * POSL:(q + 1) * POSL].astype(BF16).reshape(9, 128, POSL)
            in_maps.append({"xT": xTq, "aT": aT, "rT": rT})

    res = run_bass_kernel_spmd(nc, in_maps, core_ids=list(range(8)))

    out = np.empty((B, 64, H1, H1), np.float32)
    for s in range(B):
        Ut = np.concatenate([res.results[s * 4 + q]["uT"] for q in range(4)], 0)
        # clip correction: masked patches contribute exactly EPS_CLAMP * raw patch
        corr = EPS_CLAMP * rawRs[s][mms[s] == 0.0].sum(0, dtype=np.float64)
        y = _col2im(Ut + corr[None, :].astype(np.float32))
        for ri, r in enumerate((1, 2, 4, 8)):
            colsY = _im2col3(y, r)                     # [1152, 16384]
            o = fws[ri].reshape(16, KDIM) @ colsY + fbs[ri][:, None]
            out[s, ri * 16:(ri + 1) * 16] = np.maximum(o, 0.0).reshape(16, H1, H1)
    return out



# revision 2
# speedup vs baseline: 8.1367x; 8.1367x over previous
"""Distributed Trainium2 Bass kernel for AtnConv (contextual-attention conv).

Everything runs on device; the tunnel carries only compact inputs and the
final output. 8 cores = batch(2) x quarter(4). Within a sample group of 4:
  - x1^T and x2 (bf16, padded) are uploaded as quarter-shards and AllGathered
    device-side (HBM-HBM over NeuronLink).
  - Each core owns 1024 of the 4096 positions: scores = cols_q^T @ cols,
    scaled in f32 by SCALE*mm/norm, local softmax over all L, exact mask
    multiply + 1e-8 clamp on device.
  - U[c',pos] = R'^T Y via PE (R' streamed straight out of gathered x1^T, so
    col2im consumes U blocks per (di,dj) with no reshuffle), scatter-added
    into a 36-row window; windows AllGathered, every core assembles full y.
  - Final 4 dilated convs: 33-shift union with per-core weight data (zeros
    for foreign rates) keeps the program SPMD-uniform; each core emits only
    its rate's 16 channels [16,128,128] f32.
Host does only padding/transpose/casts and output concat.
"""

import numpy as np
import ml_dtypes

B, C, H1, H2 = 2, 128, 128, 64
L = H2 * H2            # 4096 patches / positions
POSL = 1024            # positions per core
SCALE = 10.0
EPS_NORM = 1e-4
EPS_CLAMP = 1e-8
RATES = (1, 2, 4, 8)
SHIFTS = sorted({(r * (u - 1), r * (v - 1))
                 for r in RATES for u in range(3) for v in range(3)})
NSH = len(SHIFTS)      # 33
BF16 = ml_dtypes.bfloat16
GROUPS = [[0, 1, 2, 3], [4, 5, 6, 7]]

X1CH = 130 * 130 * 128 // 4   # 540800 bf16 elems per x1 shard
X2CH = 128 * 66 * 66 // 4     # 139392 bf16 elems per x2 shard

_NC = None


def _build_nc():
    import concourse.bass as bass
    import concourse.bacc as bacc
    import concourse.mybir as mybir
    from concourse import tile

    bf = mybir.dt.bfloat16
    f32 = mybir.dt.float32
    Exp = mybir.ActivationFunctionType.Exp
    Relu = mybir.ActivationFunctionType.Relu
    X = mybir.AxisListType.X
    AG = "AllGather"
    BYP = mybir.AluOpType.bypass

    nc = bacc.Bacc(None, target_bir_lowering=False)
    p_x1 = nc.declare_dram_parameter("p_x1", [X1CH], bf, isOutput=False)
    p_x2 = nc.declare_dram_parameter("p_x2", [X2CH], bf, isOutput=False)
    p_x2q = nc.declare_dram_parameter("p_x2q", [128, 18, 66], bf, isOutput=False)
    p_sc = nc.declare_dram_parameter("p_sc", [1, L], f32, isOutput=False)
    p_mm = nc.declare_dram_parameter("p_mm", [1, L], bf, isOutput=False)
    p_fw = nc.declare_dram_parameter("p_fw", [NSH, 128, 16], bf, isOutput=False)
    p_fb = nc.declare_dram_parameter("p_fb", [16, 1], f32, isOutput=False)
    outp = nc.declare_dram_parameter("outp", [16, 128, 128], f32, isOutput=True)

    with tile.TileContext(nc) as tc:
        with (
            tc.tile_pool(name="dram", bufs=1, space="DRAM") as dram,
            tc.tile_pool(name="st", bufs=1) as st,
            tc.tile_pool(name="fin", bufs=2) as fin,
        ):
            # ---- kick off input gathers first (overlap with local prep) ----
            b_x2 = dram.tile([X2CH], bf)
            g_x2 = dram.tile([128, 66, 66], bf)
            b_x1 = dram.tile([X1CH], bf)
            g_x1 = dram.tile([130, 130, 128], bf)
            b_w = dram.tile([128, 36, 130], f32)
            g_w = dram.tile([4, 128, 36, 130], f32)
            nc.gpsimd.dma_start(b_x2[:], p_x2[:])
            nc.gpsimd.collective_compute(AG, BYP, replica_groups=GROUPS,
                                         ins=[b_x2[:]], outs=[g_x2[:]])
            nc.gpsimd.dma_start(b_x1[:], p_x1[:])
            nc.gpsimd.collective_compute(AG, BYP, replica_groups=GROUPS,
                                         ins=[b_x1[:]], outs=[g_x1[:]])

            # ---- persistent small state ----
            nbmaxs = st.tile([128, 8, 8], f32)
            rss = st.tile([128, 8, 8], f32)
            mmb = st.tile([128, L], bf)
            nc.sync.dma_start(mmb[0:1, :], p_mm[:])
            p = 1
            while p < 128:
                nc.sync.dma_start(mmb[p:2 * p, :], mmb[0:p, :])
                p *= 2

            with tc.tile_pool(name="estp", bufs=1) as estp:
                estore = estp.tile([128, 8, L], bf)   # Y^T, 64 KiB/part

                # ---- scores + block-local softmax ----
                with (
                    tc.tile_pool(name="ph1", bufs=1) as ph1,
                    tc.tile_pool(name="wka", bufs=2) as wka,
                    tc.tile_pool(name="psa", bufs=2, space=bass.MemorySpace.PSUM) as psa,
                ):
                    xt = ph1.tile([128, 9, 16, 64], bf)
                    scb = ph1.tile([128, L], f32)
                    for u in range(3):
                        for v in range(3):
                            nc.sync.dma_start(xt[:, 3 * u + v],
                                              p_x2q[:, u:u + 16, v:v + 64])
                    nc.sync.dma_start(scb[0:1, :], p_sc[:])
                    p = 1
                    while p < 128:
                        nc.sync.dma_start(scb[p:2 * p, :], scb[0:p, :])
                        p *= 2

                    for n in range(8):            # L blocks of 512 (8 i-rows)
                        a_n = wka.tile([128, 9, 8, 64], bf, tag="a_n")
                        for u in range(3):
                            for v in range(3):
                                nc.sync.dma_start(
                                    a_n[:, 3 * u + v],
                                    g_x2[:, 8 * n + u:8 * n + u + 8, v:v + 64])
                        for m in range(8):        # pos tiles of 128
                            z = psa.tile([128, 512], f32, tag="z")
                            for k in range(9):
                                nc.tensor.matmul(z[:], xt[:, k, 2 * m:2 * m + 2, :],
                                                 a_n[:, k], start=(k == 0),
                                                 stop=(k == 8))
                            zs = wka.tile([128, 512], f32, tag="zs")
                            nc.vector.tensor_mul(zs[:], z[:],
                                                 scb[:, n * 512:(n + 1) * 512])
                            nc.vector.reduce_max(nbmaxs[:, m, n:n + 1], zs[:],
                                                 axis=X, negate=True)
                            ef = wka.tile([128, 512], f32, tag="ef")
                            nc.scalar.activation(ef[:], zs[:], Exp,
                                                 bias=nbmaxs[:, m, n:n + 1],
                                                 scale=1.0)
                            nc.vector.reduce_sum(rss[:, m, n:n + 1], ef[:], axis=X)
                            nc.vector.tensor_copy(
                                estore[:, m, n * 512:(n + 1) * 512], ef[:])

                # ---- softmax finalize + exact mask & clamp ----
                for m in range(8):
                    ngm = fin.tile([128, 1], f32, tag="ngm")
                    nc.vector.tensor_reduce(ngm[:], nbmaxs[:, m, :], axis=X,
                                            op=mybir.AluOpType.min)
                    al = fin.tile([128, 8], f32, tag="al")
                    nc.scalar.activation(al[:], nbmaxs[:, m, :], Exp, bias=ngm[:],
                                         scale=-1.0)
                    pr = fin.tile([128, 8], f32, tag="pr")
                    nc.vector.tensor_mul(pr[:], al[:], rss[:, m, :])
                    sm = fin.tile([128, 1], f32, tag="sm")
                    nc.vector.reduce_sum(sm[:], pr[:], axis=X)
                    rc = fin.tile([128, 1], f32, tag="rc")
                    nc.vector.reciprocal(rc[:], sm[:])
                    be = fin.tile([128, 8], f32, tag="be")
                    nc.vector.tensor_scalar_mul(be[:], al[:], rc[:])
                    for n in range(8):
                        nc.vector.tensor_scalar_mul(
                            estore[:, m, n * 512:(n + 1) * 512],
                            estore[:, m, n * 512:(n + 1) * 512], be[:, n:n + 1])
                    nc.vector.tensor_mul(estore[:, m, :], estore[:, m, :], mmb[:])
                    nc.vector.tensor_scalar_max(estore[:, m, :], estore[:, m, :],
                                                EPS_CLAMP)

                # ---- U = R'^T Y per pos-half, col2im into window ----
                with tc.tile_pool(name="wpool", bufs=1) as wpool:
                    window = wpool.tile([128, 36, 130], f32)
                    nc.vector.memset(window[:], 0.0)
                    for half in range(2):
                        with (
                            tc.tile_pool(name="ybh", bufs=1) as ybh,
                            tc.tile_pool(name="wkc", bufs=2) as wkc,
                            tc.tile_pool(name="psb", bufs=1,
                                         space=bass.MemorySpace.PSUM) as psb,
                        ):
                            ybufT = ybh.tile([128, 32, 512], bf)
                            for mloc in range(4):
                                m = 4 * half + mloc
                                for kk in range(32):
                                    nc.sync.dma_start_transpose(
                                        ybufT[:, kk, mloc * 128:(mloc + 1) * 128],
                                        estore[:, m, kk * 128:(kk + 1) * 128])
                            for gg in range(4):
                                ups = [psb.tile([128, 8, 64], f32, tag=f"u{j}",
                                                name=f"ups{j}")
                                       for j in range(4)]
                                for k in range(32):
                                    rt = wkc.tile([128, 4, 128], bf, tag="rt")
                                    for j in range(4):
                                        g = 4 * gg + j
                                        di, dj = divmod(g, 4)
                                        nc.sync.dma_start(
                                            rt[:, j],
                                            g_x1[4 * k + di:4 * k + di + 3:2,
                                                 dj:dj + 127:2, :])
                                    for j in range(4):
                                        nc.tensor.matmul(ups[j][:], rt[:, j],
                                                         ybufT[:, k, :],
                                                         start=(k == 0),
                                                         stop=(k == 31))
                                for j in range(4):
                                    g = 4 * gg + j
                                    di, dj = divmod(g, 4)
                                    r0 = di + 1 + 16 * half
                                    sl = window[:, r0:r0 + 15:2, dj:dj + 127:2]
                                    nc.vector.tensor_add(sl, sl, ups[j][:])
                    nc.gpsimd.dma_start(b_w[:], window[:])

            # ---- gather windows, assemble y, final dilated convs ----
            nc.gpsimd.collective_compute(AG, BYP, replica_groups=GROUPS,
                                         ins=[b_w[:]], outs=[g_w[:]])
            with (
                tc.tile_pool(name="convp", bufs=1) as convp,
                tc.tile_pool(name="wkd", bufs=2) as wkd,
                tc.tile_pool(name="psc", bufs=2,
                             space=bass.MemorySpace.PSUM) as psc,
            ):
                y_bf = convp.tile([128, 144, 144], bf)
                fw_sb = convp.tile([128, NSH, 16], bf)
                fb_sb = convp.tile([16, 1], f32)
                for si in range(NSH):
                    nc.sync.dma_start(fw_sb[:, si, :], p_fw[si])
                nc.sync.dma_start(fb_sb[:], p_fb[:])
                with tc.tile_pool(name="ypool", bufs=1) as ypool:
                    y_buf = ypool.tile([128, 144, 144], f32)
                    nc.vector.memset(y_buf[:], 0.0)
                    for k in range(4):
                        wstg = wkd.tile([128, 36, 130], f32, tag="wstg")
                        nc.gpsimd.dma_start(wstg[:], g_w[k])
                        t0 = 2 if k == 0 else 1
                        t1 = 34 if k == 3 else 35
                        dst = y_buf[:, 32 * k + 6 + t0:32 * k + 6 + t1, 8:136]
                        nc.vector.tensor_add(dst, dst, wstg[:, t0:t1, 1:129])
                    nc.vector.tensor_copy(y_bf[:], y_buf[:])
                for blk in range(32):             # out row blocks of 4
                    ops = psc.tile([16, 4, 128], f32, tag="ops")
                    for si, (dh, dv) in enumerate(SHIFTS):
                        r0 = 8 + dh + 4 * blk
                        nc.tensor.matmul(ops[:], fw_sb[:, si, :],
                                         y_bf[:, r0:r0 + 4, 8 + dv:8 + dv + 128],
                                         start=(si == 0), stop=(si == NSH - 1))
                    ob = wkd.tile([16, 4, 128], f32, tag="ob")
                    nc.scalar.activation(ob[:], ops[:], Relu, bias=fb_sb[:],
                                         scale=1.0)
                    nc.sync.dma_start(outp[:, 4 * blk:4 * blk + 4, :], ob[:])
    nc.compile()
    return nc


def _get_nc():
    global _NC
    if _NC is None:
        _NC = _build_nc()
    return _NC


def _prep_sample(x1s, x2s, masks):
    """Host prep for one sample: shards + vectors (all cheap)."""
    x1tp = np.pad(x1s * 0.25, ((0, 0), (1, 1), (1, 1))).transpose(1, 2, 0)
    x1fl = np.ascontiguousarray(x1tp).astype(BF16).reshape(-1)
    x2p = np.pad(x2s, ((0, 0), (1, 1), (1, 1))).astype(BF16)
    x2fl = x2p.reshape(-1)

    sq = np.pad((x2s * x2s).sum(0), 1)
    n2 = np.zeros((H2, H2), np.float32)
    mp = np.pad(masks, 1)
    ps = np.zeros((H2, H2), np.float32)
    for u in range(3):
        for v in range(3):
            n2 += sq[u:u + H2, v:v + H2]
            ps += mp[u:u + H2, v:v + H2]
    norm = np.sqrt(n2).reshape(-1)
    mm = (ps.reshape(-1) == 0.0).astype(np.float32)
    scalev = (SCALE * mm / np.maximum(norm, EPS_NORM)).astype(np.float32)
    return x1fl, x2p, x2fl, scalev, mm


def kernel(x1, x2, mask, fw0, fb0, fw1, fb1, fw2, fb2, fw3, fb3):
    from concourse.bass_utils import run_bass_kernel_spmd

    x1 = np.asarray(x1, np.float32)
    x2 = np.asarray(x2, np.float32)
    mask = np.asarray(mask, np.float32)
    fws = [np.asarray(f, np.float32) for f in (fw0, fw1, fw2, fw3)]
    fbs = [np.asarray(f, np.float32) for f in (fb0, fb1, fb2, fb3)]

    fwt = []
    for q, r in enumerate(RATES):
        t = np.zeros((NSH, 128, 16), np.float32)
        for si, (dh, dv) in enumerate(SHIFTS):
            if dh in (-r, 0, r) and dv in (-r, 0, r):
                u, v = dh // r + 1, dv // r + 1
                t[si] = fws[q][:, :, u, v].T
        fwt.append(t.astype(BF16))

    nc = _get_nc()
    in_maps = []
    for s in range(B):
        x1fl, x2p, x2fl, scalev, mm = _prep_sample(x1[s], x2[s], mask[s, 0])
        for q in range(4):
            in_maps.append({
                "p_x1": x1fl[q * X1CH:(q + 1) * X1CH],
                "p_x2": x2fl[q * X2CH:(q + 1) * X2CH],
                "p_x2q": np.ascontiguousarray(x2p[:, 16 * q:16 * q + 18, :]),
                "p_sc": scalev[None, :],
                "p_mm": mm[None, :].astype(BF16),
                "p_fw": fwt[q],
                "p_fb": fbs[q][:, None],
            })

    res = run_bass_kernel_spmd(nc, in_maps, core_ids=list(range(8)))

    out = np.empty((B, 64, H1, H1), np.float32)
    for s in range(B):
        for q in range(4):
            out[s, 16 * q:16 * (q + 1)] = res.results[4 * s + q]["outp"]
    return out


# revision 3
# speedup vs baseline: 20.7870x; 2.5547x over previous
"""Distributed Trainium2 Bass kernel for AtnConv (contextual-attention conv).

Everything runs on device; the tunnel carries only compact inputs and the
final output. 8 cores = batch(2) x quarter(4). Within a sample group of 4:
  - x1^T and x2 (bf16, padded) are uploaded as quarter-shards and AllGathered
    device-side (HBM-HBM over NeuronLink).
  - Each core owns 1024 of the 4096 positions: scores = cols_q^T @ cols,
    scaled in f32 by SCALE*mm/norm, local softmax over all L, exact mask
    multiply + 1e-8 clamp on device.
  - U[c',pos] = R'^T Y via PE (R' streamed straight out of gathered x1^T, so
    col2im consumes U blocks per (di,dj) with no reshuffle), scatter-added
    into a 36-row window; windows AllGathered, every core assembles full y.
  - Final 4 dilated convs: 33-shift union with per-core weight data (zeros
    for foreign rates) keeps the program SPMD-uniform; each core emits only
    its rate's 16 channels [16,128,128] f32.
Host does only padding/transpose/casts and output concat.
"""

import numpy as np
import ml_dtypes


def _enable_jax_compilation_cache():
    # run_bass_kernel_spmd builds a fresh jit closure per call, so JAX's
    # in-process executable cache never hits and every dispatch re-runs the
    # BIR->NEFF compile (~0.8s). The persistent cache keys on the (stable)
    # serialized HLO and skips that.
    try:
        import jax
        jax.config.update("jax_compilation_cache_dir", "/tmp/jax_comp_cache")
        jax.config.update("jax_persistent_cache_min_compile_time_secs", 0)
        jax.config.update("jax_persistent_cache_min_entry_size_bytes", -1)
    except Exception:
        pass


_enable_jax_compilation_cache()

B, C, H1, H2 = 2, 128, 128, 64
L = H2 * H2            # 4096 patches / positions
POSL = 1024            # positions per core
SCALE = 10.0
EPS_NORM = 1e-4
EPS_CLAMP = 1e-8
RATES = (1, 2, 4, 8)
SHIFTS = sorted({(r * (u - 1), r * (v - 1))
                 for r in RATES for u in range(3) for v in range(3)})
NSH = len(SHIFTS)      # 33
BF16 = ml_dtypes.bfloat16
GROUPS = [[0, 1, 2, 3], [4, 5, 6, 7]]

X1CH = 130 * 130 * 128 // 4   # 540800 bf16 elems per x1 shard
X2CH = 128 * 66 * 66 // 4     # 139392 bf16 elems per x2 shard
# bf16 blob layout (element offsets)
OFF_X1 = 0
OFF_X2 = OFF_X1 + X1CH
OFF_X2Q = OFF_X2 + X2CH
OFF_MM = OFF_X2Q + 128 * 18 * 66
OFF_FW = OFF_MM + L
BFBLOB = OFF_FW + NSH * 128 * 16

_NC = None


def _build_nc():
    import concourse.bass as bass
    import concourse.bacc as bacc
    import concourse.mybir as mybir
    from concourse import tile

    bf = mybir.dt.bfloat16
    f32 = mybir.dt.float32
    Exp = mybir.ActivationFunctionType.Exp
    Relu = mybir.ActivationFunctionType.Relu
    X = mybir.AxisListType.X
    AG = "AllGather"
    BYP = mybir.AluOpType.bypass

    nc = bacc.Bacc(None, target_bir_lowering=False)
    p_bf = nc.declare_dram_parameter("p_bf", [BFBLOB], bf, isOutput=False)
    p_f32 = nc.declare_dram_parameter("p_f32", [L + 16], f32, isOutput=False)
    outp = nc.declare_dram_parameter("outp", [16, 128, 128], bf, isOutput=True)

    with tile.TileContext(nc) as tc:
        with (
            tc.tile_pool(name="dram", bufs=1, space="DRAM") as dram,
            tc.tile_pool(name="st", bufs=1) as st,
            tc.tile_pool(name="fin", bufs=2) as fin,
        ):
            # ---- kick off input gathers first (overlap with local prep) ----
            b_x2 = dram.tile([X2CH], bf)
            g_x2 = dram.tile([128, 66, 66], bf)
            b_x1 = dram.tile([X1CH], bf)
            g_x1 = dram.tile([130, 130, 128], bf)
            b_w = dram.tile([128, 36, 130], f32)
            g_w = dram.tile([4, 128, 36, 130], f32)
            d_x2q = dram.tile([128, 18, 66], bf)
            d_fw = dram.tile([NSH, 128, 16], bf)
            nc.gpsimd.dma_start(b_x2[:], p_bf[OFF_X2:OFF_X2 + X2CH])
            nc.gpsimd.collective_compute(AG, BYP, replica_groups=GROUPS,
                                         ins=[b_x2[:]], outs=[g_x2[:]])
            nc.gpsimd.dma_start(b_x1[:], p_bf[OFF_X1:OFF_X1 + X1CH])
            nc.gpsimd.dma_start(d_x2q[:], p_bf[OFF_X2Q:OFF_X2Q + 128 * 18 * 66])
            nc.gpsimd.dma_start(d_fw[:], p_bf[OFF_FW:OFF_FW + NSH * 128 * 16])
            nc.gpsimd.collective_compute(AG, BYP, replica_groups=GROUPS,
                                         ins=[b_x1[:]], outs=[g_x1[:]])

            # ---- persistent small state ----
            nbmaxs = st.tile([128, 8, 8], f32)
            rss = st.tile([128, 8, 8], f32)
            mmb = st.tile([128, L], bf)
            nc.sync.dma_start(mmb[0:1, :], p_bf[OFF_MM:OFF_MM + L])
            p = 1
            while p < 128:
                nc.sync.dma_start(mmb[p:2 * p, :], mmb[0:p, :])
                p *= 2

            with tc.tile_pool(name="estp", bufs=1) as estp:
                estore = estp.tile([128, 8, L], bf)   # Y^T, 64 KiB/part

                # ---- scores + block-local softmax ----
                with (
                    tc.tile_pool(name="ph1", bufs=1) as ph1,
                    tc.tile_pool(name="wka", bufs=2) as wka,
                    tc.tile_pool(name="psa", bufs=2, space=bass.MemorySpace.PSUM) as psa,
                ):
                    xt = ph1.tile([128, 9, 16, 64], bf)
                    scb = ph1.tile([128, L], f32)
                    for u in range(3):
                        for v in range(3):
                            nc.sync.dma_start(xt[:, 3 * u + v],
                                              d_x2q[:, u:u + 16, v:v + 64])
                    nc.sync.dma_start(scb[0:1, :], p_f32[0:L])
                    p = 1
                    while p < 128:
                        nc.sync.dma_start(scb[p:2 * p, :], scb[0:p, :])
                        p *= 2

                    for n in range(8):            # L blocks of 512 (8 i-rows)
                        a_n = wka.tile([128, 9, 8, 64], bf, tag="a_n")
                        for u in range(3):
                            for v in range(3):
                                nc.sync.dma_start(
                                    a_n[:, 3 * u + v],
                                    g_x2[:, 8 * n + u:8 * n + u + 8, v:v + 64])
                        for m in range(8):        # pos tiles of 128
                            z = psa.tile([128, 512], f32, tag="z")
                            for k in range(9):
                                nc.tensor.matmul(z[:], xt[:, k, 2 * m:2 * m + 2, :],
                                                 a_n[:, k], start=(k == 0),
                                                 stop=(k == 8))
                            zs = wka.tile([128, 512], f32, tag="zs")
                            nc.vector.tensor_mul(zs[:], z[:],
                                                 scb[:, n * 512:(n + 1) * 512])
                            nc.vector.reduce_max(nbmaxs[:, m, n:n + 1], zs[:],
                                                 axis=X, negate=True)
                            ef = wka.tile([128, 512], f32, tag="ef")
                            nc.scalar.activation(ef[:], zs[:], Exp,
                                                 bias=nbmaxs[:, m, n:n + 1],
                                                 scale=1.0)
                            nc.vector.reduce_sum(rss[:, m, n:n + 1], ef[:], axis=X)
                            nc.vector.tensor_copy(
                                estore[:, m, n * 512:(n + 1) * 512], ef[:])

                # ---- softmax finalize + exact mask & clamp ----
                for m in range(8):
                    ngm = fin.tile([128, 1], f32, tag="ngm")
                    nc.vector.tensor_reduce(ngm[:], nbmaxs[:, m, :], axis=X,
                                            op=mybir.AluOpType.min)
                    al = fin.tile([128, 8], f32, tag="al")
                    nc.scalar.activation(al[:], nbmaxs[:, m, :], Exp, bias=ngm[:],
                                         scale=-1.0)
                    pr = fin.tile([128, 8], f32, tag="pr")
                    nc.vector.tensor_mul(pr[:], al[:], rss[:, m, :])
                    sm = fin.tile([128, 1], f32, tag="sm")
                    nc.vector.reduce_sum(sm[:], pr[:], axis=X)
                    rc = fin.tile([128, 1], f32, tag="rc")
                    nc.vector.reciprocal(rc[:], sm[:])
                    be = fin.tile([128, 8], f32, tag="be")
                    nc.vector.tensor_scalar_mul(be[:], al[:], rc[:])
                    for n in range(8):
                        nc.vector.tensor_scalar_mul(
                            estore[:, m, n * 512:(n + 1) * 512],
                            estore[:, m, n * 512:(n + 1) * 512], be[:, n:n + 1])
                    nc.vector.tensor_mul(estore[:, m, :], estore[:, m, :], mmb[:])
                    nc.vector.tensor_scalar_max(estore[:, m, :], estore[:, m, :],
                                                EPS_CLAMP)

                # ---- U = R'^T Y per pos-half, col2im into window ----
                with tc.tile_pool(name="wpool", bufs=1) as wpool:
                    window = wpool.tile([128, 36, 130], f32)
                    nc.vector.memset(window[:], 0.0)
                    for half in range(2):
                        with (
                            tc.tile_pool(name="ybh", bufs=1) as ybh,
                            tc.tile_pool(name="wkc", bufs=2) as wkc,
                            tc.tile_pool(name="psb", bufs=1,
                                         space=bass.MemorySpace.PSUM) as psb,
                        ):
                            ybufT = ybh.tile([128, 32, 512], bf)
                            for mloc in range(4):
                                m = 4 * half + mloc
                                for kk in range(32):
                                    nc.sync.dma_start_transpose(
                                        ybufT[:, kk, mloc * 128:(mloc + 1) * 128],
                                        estore[:, m, kk * 128:(kk + 1) * 128])
                            for gg in range(4):
                                ups = [psb.tile([128, 8, 64], f32, tag=f"u{j}",
                                                name=f"ups{j}")
                                       for j in range(4)]
                                for k in range(32):
                                    rt = wkc.tile([128, 4, 128], bf, tag="rt")
                                    for j in range(4):
                                        g = 4 * gg + j
                                        di, dj = divmod(g, 4)
                                        nc.sync.dma_start(
                                            rt[:, j],
                                            g_x1[4 * k + di:4 * k + di + 3:2,
                                                 dj:dj + 127:2, :])
                                    for j in range(4):
                                        nc.tensor.matmul(ups[j][:], rt[:, j],
                                                         ybufT[:, k, :],
                                                         start=(k == 0),
                                                         stop=(k == 31))
                                for j in range(4):
                                    g = 4 * gg + j
                                    di, dj = divmod(g, 4)
                                    r0 = di + 1 + 16 * half
                                    sl = window[:, r0:r0 + 15:2, dj:dj + 127:2]
                                    nc.vector.tensor_add(sl, sl, ups[j][:])
                    nc.gpsimd.dma_start(b_w[:], window[:])

            # ---- gather windows, assemble y, final dilated convs ----
            nc.gpsimd.collective_compute(AG, BYP, replica_groups=GROUPS,
                                         ins=[b_w[:]], outs=[g_w[:]])
            with (
                tc.tile_pool(name="convp", bufs=1) as convp,
                tc.tile_pool(name="wkd", bufs=2) as wkd,
                tc.tile_pool(name="psc", bufs=2,
                             space=bass.MemorySpace.PSUM) as psc,
            ):
                y_bf = convp.tile([128, 144, 144], bf)
                fw_sb = convp.tile([128, NSH, 16], bf)
                fb_sb = convp.tile([16, 1], f32)
                for si in range(NSH):
                    nc.sync.dma_start(fw_sb[:, si, :], d_fw[si])
                nc.sync.dma_start(fb_sb[:], p_f32[L:L + 16])
                with tc.tile_pool(name="ypool", bufs=1) as ypool:
                    y_buf = ypool.tile([128, 144, 144], f32)
                    nc.vector.memset(y_buf[:], 0.0)
                    for k in range(4):
                        wstg = wkd.tile([128, 36, 130], f32, tag="wstg")
                        nc.gpsimd.dma_start(wstg[:], g_w[k])
                        t0 = 2 if k == 0 else 1
                        t1 = 34 if k == 3 else 35
                        dst = y_buf[:, 32 * k + 6 + t0:32 * k + 6 + t1, 8:136]
                        nc.vector.tensor_add(dst, dst, wstg[:, t0:t1, 1:129])
                    nc.vector.tensor_copy(y_bf[:], y_buf[:])
                for blk in range(32):             # out row blocks of 4
                    ops = psc.tile([16, 4, 128], f32, tag="ops")
                    for si, (dh, dv) in enumerate(SHIFTS):
                        r0 = 8 + dh + 4 * blk
                        nc.tensor.matmul(ops[:], fw_sb[:, si, :],
                                         y_bf[:, r0:r0 + 4, 8 + dv:8 + dv + 128],
                                         start=(si == 0), stop=(si == NSH - 1))
                    ob = wkd.tile([16, 4, 128], bf, tag="ob")
                    nc.scalar.activation(ob[:], ops[:], Relu, bias=fb_sb[:],
                                         scale=1.0)
                    nc.sync.dma_start(outp[:, 4 * blk:4 * blk + 4, :], ob[:])
    nc.compile()
    return nc


def _get_nc():
    global _NC
    if _NC is None:
        _NC = _build_nc()
    return _NC


def _prep_sample(x1s, x2s, masks):
    """Host prep for one sample: shards + vectors (all cheap)."""
    x1tp = np.pad(x1s * 0.25, ((0, 0), (1, 1), (1, 1))).transpose(1, 2, 0)
    x1fl = np.ascontiguousarray(x1tp).astype(BF16).reshape(-1)
    x2p = np.pad(x2s, ((0, 0), (1, 1), (1, 1))).astype(BF16)
    x2fl = x2p.reshape(-1)

    sq = np.pad((x2s * x2s).sum(0), 1)
    n2 = np.zeros((H2, H2), np.float32)
    mp = np.pad(masks, 1)
    ps = np.zeros((H2, H2), np.float32)
    for u in range(3):
        for v in range(3):
            n2 += sq[u:u + H2, v:v + H2]
            ps += mp[u:u + H2, v:v + H2]
    norm = np.sqrt(n2).reshape(-1)
    mm = (ps.reshape(-1) == 0.0).astype(np.float32)
    scalev = (SCALE * mm / np.maximum(norm, EPS_NORM)).astype(np.float32)
    return x1fl, x2p, x2fl, scalev, mm


def kernel(x1, x2, mask, fw0, fb0, fw1, fb1, fw2, fb2, fw3, fb3):
    from concourse.bass_utils import run_bass_kernel_spmd

    x1 = np.asarray(x1, np.float32)
    x2 = np.asarray(x2, np.float32)
    mask = np.asarray(mask, np.float32)
    fws = [np.asarray(f, np.float32) for f in (fw0, fw1, fw2, fw3)]
    fbs = [np.asarray(f, np.float32) for f in (fb0, fb1, fb2, fb3)]

    fwt = []
    for q, r in enumerate(RATES):
        t = np.zeros((NSH, 128, 16), np.float32)
        for si, (dh, dv) in enumerate(SHIFTS):
            if dh in (-r, 0, r) and dv in (-r, 0, r):
                u, v = dh // r + 1, dv // r + 1
                t[si] = fws[q][:, :, u, v].T
        fwt.append(t.astype(BF16))

    nc = _get_nc()
    in_maps = []
    for s in range(B):
        x1fl, x2p, x2fl, scalev, mm = _prep_sample(x1[s], x2[s], mask[s, 0])
        mmbf = mm.astype(BF16)
        for q in range(4):
            blob = np.concatenate([
                x1fl[q * X1CH:(q + 1) * X1CH],
                x2fl[q * X2CH:(q + 1) * X2CH],
                np.ascontiguousarray(x2p[:, 16 * q:16 * q + 18, :]).reshape(-1),
                mmbf,
                fwt[q].reshape(-1),
            ])
            in_maps.append({
                "p_bf": blob,
                "p_f32": np.concatenate([scalev, fbs[q]]),
            })

    res = run_bass_kernel_spmd(nc, in_maps, core_ids=list(range(8)))

    out = np.empty((B, 64, H1, H1), np.float32)
    for s in range(B):
        for q in range(4):
            out[s, 16 * q:16 * (q + 1)] = res.results[4 * s + q]["outp"].astype(np.float32)
    return out


# revision 4
# speedup vs baseline: 24.7422x; 1.1903x over previous
"""Distributed Trainium2 Bass kernel for AtnConv (contextual-attention conv).

Everything runs on device; the tunnel carries only compact inputs and the
final output. 8 cores = batch(2) x quarter(4). Within a sample group of 4:
  - x1^T and x2 (bf16, padded) are uploaded as quarter-shards and AllGathered
    device-side (HBM-HBM over NeuronLink).
  - Each core owns 1024 of the 4096 positions: scores = cols_q^T @ cols,
    scaled in f32 by SCALE*mm/norm, local softmax over all L, exact mask
    multiply + 1e-8 clamp on device.
  - U[c',pos] = R'^T Y via PE (R' streamed straight out of gathered x1^T, so
    col2im consumes U blocks per (di,dj) with no reshuffle), scatter-added
    into a 36-row window; windows AllGathered, every core assembles full y.
  - Final 4 dilated convs: 33-shift union with per-core weight data (zeros
    for foreign rates) keeps the program SPMD-uniform; each core emits only
    its rate's 16 channels [16,128,128] f32.
Host does only padding/transpose/casts and output concat.
"""

import numpy as np
import ml_dtypes


def _enable_jax_compilation_cache():
    # run_bass_kernel_spmd builds a fresh jit closure per call, so JAX's
    # in-process executable cache never hits and every dispatch re-runs the
    # BIR->NEFF compile (~0.8s). The persistent cache keys on the (stable)
    # serialized HLO and skips that.
    try:
        import jax
        jax.config.update("jax_compilation_cache_dir", "/tmp/jax_comp_cache")
        jax.config.update("jax_persistent_cache_min_compile_time_secs", 0)
        jax.config.update("jax_persistent_cache_min_entry_size_bytes", -1)
    except Exception:
        pass


_enable_jax_compilation_cache()

B, C, H1, H2 = 2, 128, 128, 64
L = H2 * H2            # 4096 patches / positions
POSL = 1024            # positions per core
SCALE = 10.0
EPS_NORM = 1e-4
EPS_CLAMP = 1e-8
RATES = (1, 2, 4, 8)
SHIFTS = sorted({(r * (u - 1), r * (v - 1))
                 for r in RATES for u in range(3) for v in range(3)})
NSH = len(SHIFTS)      # 33
BF16 = ml_dtypes.bfloat16
GROUPS = [[0, 1, 2, 3], [4, 5, 6, 7]]

X1CH = 130 * 130 * 128 // 4   # 540800 bf16 elems per x1 shard
X2CH = 128 * 66 * 66 // 4     # 139392 bf16 elems per x2 shard
# bf16 blob layout (element offsets)
OFF_X1 = 0
OFF_X2 = OFF_X1 + X1CH
OFF_X2Q = OFF_X2 + X2CH
OFF_MM = OFF_X2Q + 128 * 18 * 66
OFF_FW = OFF_MM + L
BFBLOB = OFF_FW + NSH * 128 * 16

_NC = None


def _build_nc():
    import concourse.bass as bass
    import concourse.bacc as bacc
    import concourse.mybir as mybir
    from concourse import tile

    bf = mybir.dt.bfloat16
    f32 = mybir.dt.float32
    Exp = mybir.ActivationFunctionType.Exp
    Relu = mybir.ActivationFunctionType.Relu
    X = mybir.AxisListType.X
    AG = "AllGather"
    BYP = mybir.AluOpType.bypass

    nc = bacc.Bacc(None, target_bir_lowering=False)
    p_bf = nc.declare_dram_parameter("p_bf", [BFBLOB], bf, isOutput=False)
    p_f32 = nc.declare_dram_parameter("p_f32", [L + 16], f32, isOutput=False)
    outp = nc.declare_dram_parameter("outp", [16, 128, 128], bf, isOutput=True)

    with tile.TileContext(nc) as tc:
        with (
            tc.tile_pool(name="dram", bufs=1, space="DRAM") as dram,
            tc.tile_pool(name="st", bufs=1) as st,
            tc.tile_pool(name="fin", bufs=2) as fin,
        ):
            # ---- kick off input gathers first (overlap with local prep) ----
            b_x2 = dram.tile([X2CH], bf)
            g_x2 = dram.tile([128, 66, 66], bf)
            b_x1 = dram.tile([X1CH], bf)
            g_x1 = dram.tile([130, 130, 128], bf)
            b_w = dram.tile([128, 36, 130], f32)
            g_w = dram.tile([4, 128, 36, 130], f32)
            d_x2q = dram.tile([128, 18, 66], bf)
            d_fw = dram.tile([NSH, 128, 16], bf)
            nc.gpsimd.dma_start(b_x2[:], p_bf[OFF_X2:OFF_X2 + X2CH])
            nc.gpsimd.collective_compute(AG, BYP, replica_groups=GROUPS,
                                         ins=[b_x2[:]], outs=[g_x2[:]])
            nc.gpsimd.dma_start(b_x1[:], p_bf[OFF_X1:OFF_X1 + X1CH])
            nc.gpsimd.dma_start(d_x2q[:], p_bf[OFF_X2Q:OFF_X2Q + 128 * 18 * 66])
            nc.gpsimd.dma_start(d_fw[:], p_bf[OFF_FW:OFF_FW + NSH * 128 * 16])
            nc.gpsimd.collective_compute(AG, BYP, replica_groups=GROUPS,
                                         ins=[b_x1[:]], outs=[g_x1[:]])

            # ---- persistent small state ----
            nbmaxs = st.tile([128, 8, 8], f32)
            rss = st.tile([128, 8, 8], f32)
            mmb = st.tile([128, L], bf)
            nc.sync.dma_start(mmb[0:1, :], p_bf[OFF_MM:OFF_MM + L])
            p = 1
            while p < 128:
                nc.sync.dma_start(mmb[p:2 * p, :], mmb[0:p, :])
                p *= 2

            with tc.tile_pool(name="estp", bufs=1) as estp:
                estore = estp.tile([128, 8, L], bf)   # Y^T, 64 KiB/part

                # ---- scores + block-local softmax ----
                with (
                    tc.tile_pool(name="ph1", bufs=1) as ph1,
                    tc.tile_pool(name="wka", bufs=2) as wka,
                    tc.tile_pool(name="psa", bufs=2, space=bass.MemorySpace.PSUM) as psa,
                ):
                    xt = ph1.tile([128, 9, 16, 64], bf)
                    scb = ph1.tile([128, L], f32)
                    for u in range(3):
                        for v in range(3):
                            nc.sync.dma_start(xt[:, 3 * u + v],
                                              d_x2q[:, u:u + 16, v:v + 64])
                    nc.sync.dma_start(scb[0:1, :], p_f32[0:L])
                    p = 1
                    while p < 128:
                        nc.sync.dma_start(scb[p:2 * p, :], scb[0:p, :])
                        p *= 2

                    for n in range(8):            # L blocks of 512 (8 i-rows)
                        a_n = wka.tile([128, 9, 8, 64], bf, tag="a_n")
                        for u in range(3):
                            for v in range(3):
                                nc.sync.dma_start(
                                    a_n[:, 3 * u + v],
                                    g_x2[:, 8 * n + u:8 * n + u + 8, v:v + 64])
                        for m in range(8):        # pos tiles of 128
                            z = psa.tile([128, 512], f32, tag="z")
                            for k in range(9):
                                nc.tensor.matmul(z[:], xt[:, k, 2 * m:2 * m + 2, :],
                                                 a_n[:, k], start=(k == 0),
                                                 stop=(k == 8))
                            zs = wka.tile([128, 512], f32, tag="zs")
                            nc.vector.tensor_mul(zs[:], z[:],
                                                 scb[:, n * 512:(n + 1) * 512])
                            nc.vector.reduce_max(nbmaxs[:, m, n:n + 1], zs[:],
                                                 axis=X, negate=True)
                            ef = wka.tile([128, 512], f32, tag="ef")
                            nc.scalar.activation(ef[:], zs[:], Exp,
                                                 bias=nbmaxs[:, m, n:n + 1],
                                                 scale=1.0)
                            nc.vector.reduce_sum(rss[:, m, n:n + 1], ef[:], axis=X)
                            nc.vector.tensor_copy(
                                estore[:, m, n * 512:(n + 1) * 512], ef[:])

                # ---- softmax finalize + exact mask & clamp ----
                for m in range(8):
                    ngm = fin.tile([128, 1], f32, tag="ngm")
                    nc.vector.tensor_reduce(ngm[:], nbmaxs[:, m, :], axis=X,
                                            op=mybir.AluOpType.min)
                    al = fin.tile([128, 8], f32, tag="al")
                    nc.scalar.activation(al[:], nbmaxs[:, m, :], Exp, bias=ngm[:],
                                         scale=-1.0)
                    pr = fin.tile([128, 8], f32, tag="pr")
                    nc.vector.tensor_mul(pr[:], al[:], rss[:, m, :])
                    sm = fin.tile([128, 1], f32, tag="sm")
                    nc.vector.reduce_sum(sm[:], pr[:], axis=X)
                    rc = fin.tile([128, 1], f32, tag="rc")
                    nc.vector.reciprocal(rc[:], sm[:])
                    be = fin.tile([128, 8], f32, tag="be")
                    nc.vector.tensor_scalar_mul(be[:], al[:], rc[:])
                    for n in range(8):
                        nc.vector.tensor_scalar_mul(
                            estore[:, m, n * 512:(n + 1) * 512],
                            estore[:, m, n * 512:(n + 1) * 512], be[:, n:n + 1])
                    nc.vector.tensor_mul(estore[:, m, :], estore[:, m, :], mmb[:])
                    nc.vector.tensor_scalar_max(estore[:, m, :], estore[:, m, :],
                                                EPS_CLAMP)

                # ---- U = R'^T Y per pos-half, col2im into window ----
                with tc.tile_pool(name="wpool", bufs=1) as wpool:
                    window = wpool.tile([128, 36, 130], f32)
                    nc.vector.memset(window[:], 0.0)
                    for half in range(2):
                        with (
                            tc.tile_pool(name="ybh", bufs=1) as ybh,
                            tc.tile_pool(name="wkc", bufs=2) as wkc,
                            tc.tile_pool(name="psb", bufs=1,
                                         space=bass.MemorySpace.PSUM) as psb,
                        ):
                            ybufT = ybh.tile([128, 32, 512], bf)
                            for mloc in range(4):
                                m = 4 * half + mloc
                                for kk in range(32):
                                    nc.sync.dma_start_transpose(
                                        ybufT[:, kk, mloc * 128:(mloc + 1) * 128],
                                        estore[:, m, kk * 128:(kk + 1) * 128])
                            for gg in range(4):
                                ups = [psb.tile([128, 8, 64], f32, tag=f"u{j}",
                                                name=f"ups{j}")
                                       for j in range(4)]
                                for k in range(32):
                                    rt = wkc.tile([128, 4, 128], bf, tag="rt")
                                    for j in range(4):
                                        g = 4 * gg + j
                                        di, dj = divmod(g, 4)
                                        nc.sync.dma_start(
                                            rt[:, j],
                                            g_x1[4 * k + di:4 * k + di + 3:2,
                                                 dj:dj + 127:2, :])
                                    for j in range(4):
                                        nc.tensor.matmul(ups[j][:], rt[:, j],
                                                         ybufT[:, k, :],
                                                         start=(k == 0),
                                                         stop=(k == 31))
                                for j in range(4):
                                    g = 4 * gg + j
                                    di, dj = divmod(g, 4)
                                    r0 = di + 1 + 16 * half
                                    sl = window[:, r0:r0 + 15:2, dj:dj + 127:2]
                                    nc.vector.tensor_add(sl, sl, ups[j][:])
                    nc.gpsimd.dma_start(b_w[:], window[:])

            # ---- gather windows, assemble y, final dilated convs ----
            nc.gpsimd.collective_compute(AG, BYP, replica_groups=GROUPS,
                                         ins=[b_w[:]], outs=[g_w[:]])
            with (
                tc.tile_pool(name="convp", bufs=1) as convp,
                tc.tile_pool(name="wkd", bufs=2) as wkd,
                tc.tile_pool(name="psc", bufs=2,
                             space=bass.MemorySpace.PSUM) as psc,
            ):
                y_bf = convp.tile([128, 144, 144], bf)
                fw_sb = convp.tile([128, NSH, 16], bf)
                fb_sb = convp.tile([16, 1], f32)
                for si in range(NSH):
                    nc.sync.dma_start(fw_sb[:, si, :], d_fw[si])
                nc.sync.dma_start(fb_sb[:], p_f32[L:L + 16])
                with tc.tile_pool(name="ypool", bufs=1) as ypool:
                    y_buf = ypool.tile([128, 144, 144], f32)
                    nc.vector.memset(y_buf[:], 0.0)
                    for k in range(4):
                        wstg = wkd.tile([128, 36, 130], f32, tag="wstg")
                        nc.gpsimd.dma_start(wstg[:], g_w[k])
                        t0 = 2 if k == 0 else 1
                        t1 = 34 if k == 3 else 35
                        dst = y_buf[:, 32 * k + 6 + t0:32 * k + 6 + t1, 8:136]
                        nc.vector.tensor_add(dst, dst, wstg[:, t0:t1, 1:129])
                    nc.vector.tensor_copy(y_bf[:], y_buf[:])
                for blk in range(32):             # out row blocks of 4
                    ops = psc.tile([16, 4, 128], f32, tag="ops")
                    for si, (dh, dv) in enumerate(SHIFTS):
                        r0 = 8 + dh + 4 * blk
                        nc.tensor.matmul(ops[:], fw_sb[:, si, :],
                                         y_bf[:, r0:r0 + 4, 8 + dv:8 + dv + 128],
                                         start=(si == 0), stop=(si == NSH - 1))
                    ob = wkd.tile([16, 4, 128], bf, tag="ob")
                    nc.scalar.activation(ob[:], ops[:], Relu, bias=fb_sb[:],
                                         scale=1.0)
                    nc.sync.dma_start(outp[:, 4 * blk:4 * blk + 4, :], ob[:])
    nc.compile()
    return nc


def _get_nc():
    global _NC
    if _NC is None:
        _NC = _build_nc()
        # The custom-call lowering re-serializes the (immutable, already
        # compiled) BIR module on every dispatch (~60ms); serve it cached.
        blob = _NC.to_json_bytes()
        _NC.to_json_bytes = lambda: blob
    return _NC


def _prep_sample(x1s, x2s, masks):
    """Host prep for one sample: shards + vectors (all cheap)."""
    x1tp = np.pad(x1s * 0.25, ((0, 0), (1, 1), (1, 1))).transpose(1, 2, 0)
    x1fl = np.ascontiguousarray(x1tp).astype(BF16).reshape(-1)
    x2p = np.pad(x2s, ((0, 0), (1, 1), (1, 1))).astype(BF16)
    x2fl = x2p.reshape(-1)

    sq = np.pad((x2s * x2s).sum(0), 1)
    n2 = np.zeros((H2, H2), np.float32)
    mp = np.pad(masks, 1)
    ps = np.zeros((H2, H2), np.float32)
    for u in range(3):
        for v in range(3):
            n2 += sq[u:u + H2, v:v + H2]
            ps += mp[u:u + H2, v:v + H2]
    norm = np.sqrt(n2).reshape(-1)
    mm = (ps.reshape(-1) == 0.0).astype(np.float32)
    scalev = (SCALE * mm / np.maximum(norm, EPS_NORM)).astype(np.float32)
    return x1fl, x2p, x2fl, scalev, mm


def kernel(x1, x2, mask, fw0, fb0, fw1, fb1, fw2, fb2, fw3, fb3):
    from concourse.bass_utils import run_bass_kernel_spmd

    x1 = np.asarray(x1, np.float32)
    x2 = np.asarray(x2, np.float32)
    mask = np.asarray(mask, np.float32)
    fws = [np.asarray(f, np.float32) for f in (fw0, fw1, fw2, fw3)]
    fbs = [np.asarray(f, np.float32) for f in (fb0, fb1, fb2, fb3)]

    fwt = []
    for q, r in enumerate(RATES):
        t = np.zeros((NSH, 128, 16), np.float32)
        for si, (dh, dv) in enumerate(SHIFTS):
            if dh in (-r, 0, r) and dv in (-r, 0, r):
                u, v = dh // r + 1, dv // r + 1
                t[si] = fws[q][:, :, u, v].T
        fwt.append(t.astype(BF16))

    nc = _get_nc()
    in_maps = []
    for s in range(B):
        x1fl, x2p, x2fl, scalev, mm = _prep_sample(x1[s], x2[s], mask[s, 0])
        mmbf = mm.astype(BF16)
        for q in range(4):
            blob = np.concatenate([
                x1fl[q * X1CH:(q + 1) * X1CH],
                x2fl[q * X2CH:(q + 1) * X2CH],
                np.ascontiguousarray(x2p[:, 16 * q:16 * q + 18, :]).reshape(-1),
                mmbf,
                fwt[q].reshape(-1),
            ])
            in_maps.append({
                "p_bf": blob,
                "p_f32": np.concatenate([scalev, fbs[q]]),
            })

    res = run_bass_kernel_spmd(nc, in_maps, core_ids=list(range(8)))

    out = np.empty((B, 64, H1, H1), np.float32)
    for s in range(B):
        for q in range(4):
            out[s, 16 * q:16 * (q + 1)] = res.results[4 * s + q]["outp"].astype(np.float32)
    return out


# revision 5
# speedup vs baseline: 28.2591x; 1.1421x over previous
"""Distributed Trainium2 Bass kernel for AtnConv (contextual-attention conv).

Everything runs on device; the tunnel carries only compact inputs and the
final output. 8 cores = batch(2) x quarter(4). Within a sample group of 4:
  - x1^T and x2 (bf16, padded) are uploaded as quarter-shards and AllGathered
    device-side (HBM-HBM over NeuronLink).
  - Each core owns 1024 of the 4096 positions: scores = cols_q^T @ cols,
    scaled in f32 by SCALE*mm/norm, local softmax over all L, exact mask
    multiply + 1e-8 clamp on device.
  - U[c',pos] = R'^T Y via PE (R' streamed straight out of gathered x1^T, so
    col2im consumes U blocks per (di,dj) with no reshuffle), scatter-added
    into a 36-row window; windows AllGathered, every core assembles full y.
  - Final 4 dilated convs: 33-shift union with per-core weight data (zeros
    for foreign rates) keeps the program SPMD-uniform; each core emits only
    its rate's 16 channels [16,128,128] f32.
Host does only padding/transpose/casts and output concat.
"""

import numpy as np
import ml_dtypes


def _enable_jax_compilation_cache():
    # run_bass_kernel_spmd builds a fresh jit closure per call, so JAX's
    # in-process executable cache never hits and every dispatch re-runs the
    # BIR->NEFF compile (~0.8s). The persistent cache keys on the (stable)
    # serialized HLO and skips that.
    try:
        import jax
        jax.config.update("jax_compilation_cache_dir", "/tmp/jax_comp_cache")
        jax.config.update("jax_persistent_cache_min_compile_time_secs", 0)
        jax.config.update("jax_persistent_cache_min_entry_size_bytes", -1)
    except Exception:
        pass


_enable_jax_compilation_cache()

B, C, H1, H2 = 2, 128, 128, 64
L = H2 * H2            # 4096 patches / positions
POSL = 1024            # positions per core
SCALE = 10.0
EPS_NORM = 1e-4
EPS_CLAMP = 1e-8
RATES = (1, 2, 4, 8)
SHIFTS = sorted({(r * (u - 1), r * (v - 1))
                 for r in RATES for u in range(3) for v in range(3)})
NSH = len(SHIFTS)      # 33
BF16 = ml_dtypes.bfloat16
GROUPS = [[0, 1, 2, 3], [4, 5, 6, 7]]

X1CH = 130 * 130 * 128 // 4   # 540800 bf16 elems per x1 shard
X2QCH = 128 * 18 * 66         # one overlapping 18-row x2 chunk (halo 1)
# bf16 blob layout (element offsets)
OFF_X1 = 0
OFF_X2Q = OFF_X1 + X1CH
OFF_MM = OFF_X2Q + X2QCH
OFF_FW = OFF_MM + L
BFBLOB = OFF_FW + NSH * 128 * 16

_NC = None


def _build_nc():
    import concourse.bass as bass
    import concourse.bacc as bacc
    import concourse.mybir as mybir
    from concourse import tile

    bf = mybir.dt.bfloat16
    f32 = mybir.dt.float32
    Exp = mybir.ActivationFunctionType.Exp
    Relu = mybir.ActivationFunctionType.Relu
    X = mybir.AxisListType.X
    AG = "AllGather"
    BYP = mybir.AluOpType.bypass

    nc = bacc.Bacc(None, target_bir_lowering=False)
    p_bf = nc.declare_dram_parameter("p_bf", [BFBLOB], bf, isOutput=False)
    p_f32 = nc.declare_dram_parameter("p_f32", [L + 16], f32, isOutput=False)
    outp = nc.declare_dram_parameter("outp", [16, 128, 128], bf, isOutput=True)

    with tile.TileContext(nc) as tc:
        with (
            tc.tile_pool(name="dram", bufs=1, space="DRAM") as dram,
            tc.tile_pool(name="st", bufs=1) as st,
            tc.tile_pool(name="fin", bufs=2) as fin,
        ):
            # ---- kick off input gathers first (overlap with local prep) ----
            b_x2q = dram.tile([128, 18, 66], bf)
            g_x2q = dram.tile([4, 128, 18, 66], bf)
            b_x1 = dram.tile([X1CH], bf)
            g_x1 = dram.tile([130, 130, 128], bf)
            b_w = dram.tile([128, 36, 130], f32)
            g_w = dram.tile([4, 128, 36, 130], f32)
            d_fw = dram.tile([NSH, 128, 16], bf)
            nc.gpsimd.dma_start(b_x2q[:], p_bf[OFF_X2Q:OFF_X2Q + X2QCH])
            nc.gpsimd.collective_compute(AG, BYP, replica_groups=GROUPS,
                                         ins=[b_x2q[:]], outs=[g_x2q[:]])
            nc.gpsimd.dma_start(b_x1[:], p_bf[OFF_X1:OFF_X1 + X1CH])
            nc.gpsimd.dma_start(d_fw[:], p_bf[OFF_FW:OFF_FW + NSH * 128 * 16])
            nc.gpsimd.collective_compute(AG, BYP, replica_groups=GROUPS,
                                         ins=[b_x1[:]], outs=[g_x1[:]])

            # ---- persistent small state ----
            nbmaxs = st.tile([128, 8, 8], f32)
            rss = st.tile([128, 8, 8], f32)
            mmb = st.tile([128, L], bf)
            nc.sync.dma_start(mmb[0:1, :], p_bf[OFF_MM:OFF_MM + L])
            p = 1
            while p < 128:
                nc.sync.dma_start(mmb[p:2 * p, :], mmb[0:p, :])
                p *= 2

            with tc.tile_pool(name="estp", bufs=1) as estp:
                estore = estp.tile([128, 8, L], bf)   # Y^T, 64 KiB/part

                # ---- scores + block-local softmax ----
                with (
                    tc.tile_pool(name="ph1", bufs=1) as ph1,
                    tc.tile_pool(name="wka", bufs=2) as wka,
                    tc.tile_pool(name="psa", bufs=2, space=bass.MemorySpace.PSUM) as psa,
                ):
                    xt = ph1.tile([128, 9, 16, 64], bf)
                    scb = ph1.tile([128, L], f32)
                    for u in range(3):
                        for v in range(3):
                            nc.sync.dma_start(xt[:, 3 * u + v],
                                              b_x2q[:, u:u + 16, v:v + 64])
                    nc.sync.dma_start(scb[0:1, :], p_f32[0:L])
                    p = 1
                    while p < 128:
                        nc.sync.dma_start(scb[p:2 * p, :], scb[0:p, :])
                        p *= 2

                    for n in range(8):            # L blocks of 512 (8 i-rows)
                        a_n = wka.tile([128, 9, 8, 64], bf, tag="a_n")
                        ch = n // 2
                        r0 = 8 * n - 16 * ch
                        for u in range(3):
                            for v in range(3):
                                nc.sync.dma_start(
                                    a_n[:, 3 * u + v],
                                    g_x2q[ch][:, r0 + u:r0 + u + 8, v:v + 64])
                        for m in range(8):        # pos tiles of 128
                            z = psa.tile([128, 512], f32, tag="z")
                            for k in range(9):
                                nc.tensor.matmul(z[:], xt[:, k, 2 * m:2 * m + 2, :],
                                                 a_n[:, k], start=(k == 0),
                                                 stop=(k == 8))
                            zs = wka.tile([128, 512], f32, tag="zs")
                            nc.vector.tensor_mul(zs[:], z[:],
                                                 scb[:, n * 512:(n + 1) * 512])
                            nc.vector.reduce_max(nbmaxs[:, m, n:n + 1], zs[:],
                                                 axis=X, negate=True)
                            ef = wka.tile([128, 512], f32, tag="ef")
                            nc.scalar.activation(ef[:], zs[:], Exp,
                                                 bias=nbmaxs[:, m, n:n + 1],
                                                 scale=1.0)
                            nc.vector.reduce_sum(rss[:, m, n:n + 1], ef[:], axis=X)
                            nc.vector.tensor_copy(
                                estore[:, m, n * 512:(n + 1) * 512], ef[:])

                # ---- softmax finalize + exact mask & clamp ----
                for m in range(8):
                    ngm = fin.tile([128, 1], f32, tag="ngm")
                    nc.vector.tensor_reduce(ngm[:], nbmaxs[:, m, :], axis=X,
                                            op=mybir.AluOpType.min)
                    al = fin.tile([128, 8], f32, tag="al")
                    nc.scalar.activation(al[:], nbmaxs[:, m, :], Exp, bias=ngm[:],
                                         scale=-1.0)
                    pr = fin.tile([128, 8], f32, tag="pr")
                    nc.vector.tensor_mul(pr[:], al[:], rss[:, m, :])
                    sm = fin.tile([128, 1], f32, tag="sm")
                    nc.vector.reduce_sum(sm[:], pr[:], axis=X)
                    rc = fin.tile([128, 1], f32, tag="rc")
                    nc.vector.reciprocal(rc[:], sm[:])
                    be = fin.tile([128, 8], f32, tag="be")
                    nc.vector.tensor_scalar_mul(be[:], al[:], rc[:])
                    for n in range(8):
                        nc.vector.tensor_scalar_mul(
                            estore[:, m, n * 512:(n + 1) * 512],
                            estore[:, m, n * 512:(n + 1) * 512], be[:, n:n + 1])
                    nc.vector.tensor_mul(estore[:, m, :], estore[:, m, :], mmb[:])
                    nc.vector.tensor_scalar_max(estore[:, m, :], estore[:, m, :],
                                                EPS_CLAMP)

                # ---- U = R'^T Y per pos-half, col2im into window ----
                with tc.tile_pool(name="wpool", bufs=1) as wpool:
                    window = wpool.tile([128, 36, 130], f32)
                    nc.vector.memset(window[:], 0.0)
                    for half in range(2):
                        with (
                            tc.tile_pool(name="ybh", bufs=1) as ybh,
                            tc.tile_pool(name="wkc", bufs=2) as wkc,
                            tc.tile_pool(name="psb", bufs=1,
                                         space=bass.MemorySpace.PSUM) as psb,
                        ):
                            ybufT = ybh.tile([128, 32, 512], bf)
                            for mloc in range(4):
                                m = 4 * half + mloc
                                for kk in range(32):
                                    nc.sync.dma_start_transpose(
                                        ybufT[:, kk, mloc * 128:(mloc + 1) * 128],
                                        estore[:, m, kk * 128:(kk + 1) * 128])
                            for gg in range(4):
                                ups = [psb.tile([128, 8, 64], f32, tag=f"u{j}",
                                                name=f"ups{j}")
                                       for j in range(4)]
                                for k in range(32):
                                    rt = wkc.tile([128, 4, 128], bf, tag="rt")
                                    for j in range(4):
                                        g = 4 * gg + j
                                        di, dj = divmod(g, 4)
                                        nc.sync.dma_start(
                                            rt[:, j],
                                            g_x1[4 * k + di:4 * k + di + 3:2,
                                                 dj:dj + 127:2, :])
                                    for j in range(4):
                                        nc.tensor.matmul(ups[j][:], rt[:, j],
                                                         ybufT[:, k, :],
                                                         start=(k == 0),
                                                         stop=(k == 31))
                                for j in range(4):
                                    g = 4 * gg + j
                                    di, dj = divmod(g, 4)
                                    r0 = di + 1 + 16 * half
                                    sl = window[:, r0:r0 + 15:2, dj:dj + 127:2]
                                    nc.vector.tensor_add(sl, sl, ups[j][:])
                    nc.gpsimd.dma_start(b_w[:], window[:])

            # ---- gather windows, assemble y, final dilated convs ----
            nc.gpsimd.collective_compute(AG, BYP, replica_groups=GROUPS,
                                         ins=[b_w[:]], outs=[g_w[:]])
            with (
                tc.tile_pool(name="convp", bufs=1) as convp,
                tc.tile_pool(name="wkd", bufs=2) as wkd,
                tc.tile_pool(name="psc", bufs=2,
                             space=bass.MemorySpace.PSUM) as psc,
            ):
                y_bf = convp.tile([128, 144, 144], bf)
                fw_sb = convp.tile([128, NSH, 16], bf)
                fb_sb = convp.tile([16, 1], f32)
                for si in range(NSH):
                    nc.sync.dma_start(fw_sb[:, si, :], d_fw[si])
                nc.sync.dma_start(fb_sb[:], p_f32[L:L + 16])
                with tc.tile_pool(name="ypool", bufs=1) as ypool:
                    y_buf = ypool.tile([128, 144, 144], f32)
                    nc.vector.memset(y_buf[:], 0.0)
                    for k in range(4):
                        wstg = wkd.tile([128, 36, 130], f32, tag="wstg")
                        nc.gpsimd.dma_start(wstg[:], g_w[k])
                        t0 = 2 if k == 0 else 1
                        t1 = 34 if k == 3 else 35
                        dst = y_buf[:, 32 * k + 6 + t0:32 * k + 6 + t1, 8:136]
                        nc.vector.tensor_add(dst, dst, wstg[:, t0:t1, 1:129])
                    nc.vector.tensor_copy(y_bf[:], y_buf[:])
                for blk in range(32):             # out row blocks of 4
                    ops = psc.tile([16, 4, 128], f32, tag="ops")
                    for si, (dh, dv) in enumerate(SHIFTS):
                        r0 = 8 + dh + 4 * blk
                        nc.tensor.matmul(ops[:], fw_sb[:, si, :],
                                         y_bf[:, r0:r0 + 4, 8 + dv:8 + dv + 128],
                                         start=(si == 0), stop=(si == NSH - 1))
                    ob = wkd.tile([16, 4, 128], bf, tag="ob")
                    nc.scalar.activation(ob[:], ops[:], Relu, bias=fb_sb[:],
                                         scale=1.0)
                    nc.sync.dma_start(outp[:, 4 * blk:4 * blk + 4, :], ob[:])
    nc.compile()
    return nc


def _get_nc():
    global _NC
    if _NC is None:
        _NC = _build_nc()
        # The custom-call lowering re-serializes the (immutable, already
        # compiled) BIR module on every dispatch (~60ms); serve it cached.
        blob = _NC.to_json_bytes()
        _NC.to_json_bytes = lambda: blob
    return _NC


def _prep_sample(x1s, x2s, masks):
    """Host prep for one sample: shards + vectors (all cheap)."""
    x1tp = np.pad(x1s * 0.25, ((0, 0), (1, 1), (1, 1))).transpose(1, 2, 0)
    x1fl = np.ascontiguousarray(x1tp).astype(BF16).reshape(-1)
    x2p = np.pad(x2s, ((0, 0), (1, 1), (1, 1))).astype(BF16)

    sq = np.pad((x2s * x2s).sum(0), 1)
    n2 = np.zeros((H2, H2), np.float32)
    mp = np.pad(masks, 1)
    ps = np.zeros((H2, H2), np.float32)
    for u in range(3):
        for v in range(3):
            n2 += sq[u:u + H2, v:v + H2]
            ps += mp[u:u + H2, v:v + H2]
    norm = np.sqrt(n2).reshape(-1)
    mm = (ps.reshape(-1) == 0.0).astype(np.float32)
    scalev = (SCALE * mm / np.maximum(norm, EPS_NORM)).astype(np.float32)
    return x1fl, x2p, scalev, mm


def kernel(x1, x2, mask, fw0, fb0, fw1, fb1, fw2, fb2, fw3, fb3):
    from concourse.bass_utils import run_bass_kernel_spmd

    x1 = np.asarray(x1, np.float32)
    x2 = np.asarray(x2, np.float32)
    mask = np.asarray(mask, np.float32)
    fws = [np.asarray(f, np.float32) for f in (fw0, fw1, fw2, fw3)]
    fbs = [np.asarray(f, np.float32) for f in (fb0, fb1, fb2, fb3)]

    fwt = []
    for q, r in enumerate(RATES):
        t = np.zeros((NSH, 128, 16), np.float32)
        for si, (dh, dv) in enumerate(SHIFTS):
            if dh in (-r, 0, r) and dv in (-r, 0, r):
                u, v = dh // r + 1, dv // r + 1
                t[si] = fws[q][:, :, u, v].T
        fwt.append(t.astype(BF16))

    nc = _get_nc()
    in_maps = []
    for s in range(B):
        x1fl, x2p, scalev, mm = _prep_sample(x1[s], x2[s], mask[s, 0])
        mmbf = mm.astype(BF16)
        for q in range(4):
            blob = np.concatenate([
                x1fl[q * X1CH:(q + 1) * X1CH],
                np.ascontiguousarray(x2p[:, 16 * q:16 * q + 18, :]).reshape(-1),
                mmbf,
                fwt[q].reshape(-1),
            ])
            in_maps.append({
                "p_bf": blob,
                "p_f32": np.concatenate([scalev, fbs[q]]),
            })

    res = run_bass_kernel_spmd(nc, in_maps, core_ids=list(range(8)))

    out = np.empty((B, 64, H1, H1), np.float32)
    for s in range(B):
        for q in range(4):
            out[s, 16 * q:16 * (q + 1)] = res.results[4 * s + q]["outp"].astype(np.float32)
    return out


# revision 6
# speedup vs baseline: 31.1548x; 1.1025x over previous
"""Distributed Trainium2 Bass kernel for AtnConv (contextual-attention conv).

Everything runs on device; the tunnel carries only compact inputs and the
final output. 8 cores = batch(2) x quarter(4). Within a sample group of 4:
  - x1^T and x2 (bf16, padded) are uploaded as quarter-shards and AllGathered
    device-side (HBM-HBM over NeuronLink).
  - Each core owns 1024 of the 4096 positions: scores = cols_q^T @ cols,
    scaled in f32 by SCALE*mm/norm, local softmax over all L, exact mask
    multiply + 1e-8 clamp on device.
  - U[c',pos] = R'^T Y via PE (R' streamed straight out of gathered x1^T, so
    col2im consumes U blocks per (di,dj) with no reshuffle), scatter-added
    into a 36-row window; windows AllGathered, every core assembles full y.
  - Final 4 dilated convs: 33-shift union with per-core weight data (zeros
    for foreign rates) keeps the program SPMD-uniform; each core emits only
    its rate's 16 channels [16,128,128] bf16.
Host does only padding/transpose/casts and output concat.
"""

import numpy as np
import ml_dtypes


def _enable_jax_compilation_cache():
    # run_bass_kernel_spmd builds a fresh jit closure per call, so JAX's
    # in-process executable cache never hits and every dispatch re-runs the
    # BIR->NEFF compile (~0.8s). The persistent cache keys on the (stable)
    # serialized HLO and skips that.
    try:
        import jax
        jax.config.update("jax_compilation_cache_dir", "/tmp/jax_comp_cache")
        jax.config.update("jax_persistent_cache_min_compile_time_secs", 0)
        jax.config.update("jax_persistent_cache_min_entry_size_bytes", -1)
    except Exception:
        pass


_enable_jax_compilation_cache()

B, C, H1, H2 = 2, 128, 128, 64
L = H2 * H2            # 4096 patches / positions
POSL = 1024            # positions per core
SCALE = 10.0
EPS_NORM = 1e-4
EPS_CLAMP = 1e-8
RATES = (1, 2, 4, 8)
SHIFTS = sorted({(r * (u - 1), r * (v - 1))
                 for r in RATES for u in range(3) for v in range(3)})
NSH = len(SHIFTS)      # 33
BF16 = ml_dtypes.bfloat16
GROUPS = [[0, 1, 2, 3], [4, 5, 6, 7]]

X1CH = 130 * 130 * 128 // 4   # 540800 bf16 elems per x1 shard
X2QCH = 128 * 18 * 66         # one overlapping 18-row x2 chunk (halo 1)
# bf16 blob layout (element offsets)
OFF_X1 = 0
OFF_X2Q = OFF_X1 + X1CH
OFF_MM = OFF_X2Q + X2QCH
OFF_FW = OFF_MM + L
BFBLOB = OFF_FW + NSH * 128 * 16

_NC = None


def _build_nc():
    import concourse.bass as bass
    import concourse.bacc as bacc
    import concourse.mybir as mybir
    from concourse import tile

    bf = mybir.dt.bfloat16
    f32 = mybir.dt.float32
    Exp = mybir.ActivationFunctionType.Exp
    Relu = mybir.ActivationFunctionType.Relu
    X = mybir.AxisListType.X
    AG = "AllGather"
    BYP = mybir.AluOpType.bypass

    nc = bacc.Bacc(None, target_bir_lowering=False)
    p_bf = nc.declare_dram_parameter("p_bf", [BFBLOB], bf, isOutput=False)
    p_f32 = nc.declare_dram_parameter("p_f32", [L + 16], f32, isOutput=False)
    outp = nc.declare_dram_parameter("outp", [16, 128, 128], bf, isOutput=True)

    with tile.TileContext(nc) as tc:
        with (
            tc.tile_pool(name="dram", bufs=1, space="DRAM") as dram,
            tc.tile_pool(name="st", bufs=1) as st,
            tc.tile_pool(name="fin", bufs=2) as fin,
        ):
            # ---- kick off input gathers first (overlap with local prep) ----
            b_x2q = dram.tile([128, 18, 66], bf)
            g_x2q = dram.tile([4, 128, 18, 66], bf)
            b_x1 = dram.tile([X1CH], bf)
            g_x1 = dram.tile([130, 130, 128], bf)
            b_w = dram.tile([128, 36, 130], f32)
            g_w = dram.tile([4, 128, 36, 130], f32)
            d_fw = dram.tile([NSH, 128, 16], bf)
            nc.gpsimd.dma_start(b_x2q[:], p_bf[OFF_X2Q:OFF_X2Q + X2QCH])
            nc.gpsimd.collective_compute(AG, BYP, replica_groups=GROUPS,
                                         ins=[b_x2q[:]], outs=[g_x2q[:]])
            nc.gpsimd.dma_start(b_x1[:], p_bf[OFF_X1:OFF_X1 + X1CH])
            nc.gpsimd.dma_start(d_fw[:], p_bf[OFF_FW:OFF_FW + NSH * 128 * 16])
            nc.gpsimd.collective_compute(AG, BYP, replica_groups=GROUPS,
                                         ins=[b_x1[:]], outs=[g_x1[:]])

            # ---- persistent small state ----
            nbmaxs = st.tile([128, 8, 8], f32)
            rss = st.tile([128, 8, 8], f32)
            mmb = st.tile([128, L], bf)
            nc.sync.dma_start(mmb[0:1, :], p_bf[OFF_MM:OFF_MM + L])
            p = 1
            while p < 128:
                nc.sync.dma_start(mmb[p:2 * p, :], mmb[0:p, :])
                p *= 2

            with tc.tile_pool(name="estp", bufs=1) as estp:
                estore = estp.tile([128, 8, L], bf)   # Y^T, 64 KiB/part

                # ---- scores + block-local softmax ----
                with (
                    tc.tile_pool(name="ph1", bufs=1) as ph1,
                    tc.tile_pool(name="wka", bufs=2) as wka,
                    tc.tile_pool(name="psa", bufs=2, space=bass.MemorySpace.PSUM) as psa,
                ):
                    xt = ph1.tile([128, 9, 16, 64], bf)
                    scb = ph1.tile([128, L], f32)
                    for u in range(3):
                        for v in range(3):
                            nc.sync.dma_start(xt[:, 3 * u + v],
                                              b_x2q[:, u:u + 16, v:v + 64])
                    nc.sync.dma_start(scb[0:1, :], p_f32[0:L])
                    p = 1
                    while p < 128:
                        nc.sync.dma_start(scb[p:2 * p, :], scb[0:p, :])
                        p *= 2

                    for n in range(8):            # L blocks of 512 (8 i-rows)
                        a_n = wka.tile([128, 9, 8, 64], bf, tag="a_n")
                        ch = n // 2
                        r0 = 8 * n - 16 * ch
                        for u in range(3):
                            for v in range(3):
                                nc.sync.dma_start(
                                    a_n[:, 3 * u + v],
                                    g_x2q[ch][:, r0 + u:r0 + u + 8, v:v + 64])
                        for m in range(8):        # pos tiles of 128
                            z = psa.tile([128, 512], f32, tag="z")
                            for k in range(9):
                                nc.tensor.matmul(z[:], xt[:, k, 2 * m:2 * m + 2, :],
                                                 a_n[:, k], start=(k == 0),
                                                 stop=(k == 8))
                            zs = wka.tile([128, 512], f32, tag="zs")
                            nc.vector.tensor_mul(zs[:], z[:],
                                                 scb[:, n * 512:(n + 1) * 512])
                            nc.vector.reduce_max(nbmaxs[:, m, n:n + 1], zs[:],
                                                 axis=X, negate=True)
                            ef = wka.tile([128, 512], f32, tag="ef")
                            nc.scalar.activation(ef[:], zs[:], Exp,
                                                 bias=nbmaxs[:, m, n:n + 1],
                                                 scale=1.0)
                            nc.vector.reduce_sum(rss[:, m, n:n + 1], ef[:], axis=X)
                            nc.vector.tensor_copy(
                                estore[:, m, n * 512:(n + 1) * 512], ef[:])

                # ---- softmax finalize + exact mask & clamp ----
                for m in range(8):
                    ngm = fin.tile([128, 1], f32, tag="ngm")
                    nc.vector.tensor_reduce(ngm[:], nbmaxs[:, m, :], axis=X,
                                            op=mybir.AluOpType.min)
                    al = fin.tile([128, 8], f32, tag="al")
                    nc.scalar.activation(al[:], nbmaxs[:, m, :], Exp, bias=ngm[:],
                                         scale=-1.0)
                    pr = fin.tile([128, 8], f32, tag="pr")
                    nc.vector.tensor_mul(pr[:], al[:], rss[:, m, :])
                    sm = fin.tile([128, 1], f32, tag="sm")
                    nc.vector.reduce_sum(sm[:], pr[:], axis=X)
                    rc = fin.tile([128, 1], f32, tag="rc")
                    nc.vector.reciprocal(rc[:], sm[:])
                    be = fin.tile([128, 8], f32, tag="be")
                    nc.vector.tensor_scalar_mul(be[:], al[:], rc[:])
                    for n in range(8):
                        nc.vector.tensor_scalar_mul(
                            estore[:, m, n * 512:(n + 1) * 512],
                            estore[:, m, n * 512:(n + 1) * 512], be[:, n:n + 1])
                    nc.vector.tensor_mul(estore[:, m, :], estore[:, m, :], mmb[:])
                    nc.vector.tensor_scalar_max(estore[:, m, :], estore[:, m, :],
                                                EPS_CLAMP)

                # ---- U = R'^T Y per pos-half, col2im into window ----
                with tc.tile_pool(name="wpool", bufs=1) as wpool:
                    window = wpool.tile([128, 36, 130], f32)
                    nc.vector.memset(window[:], 0.0)
                    for half in range(2):
                        with (
                            tc.tile_pool(name="ybh", bufs=1) as ybh,
                            tc.tile_pool(name="wkc", bufs=2) as wkc,
                            tc.tile_pool(name="psb", bufs=1,
                                         space=bass.MemorySpace.PSUM) as psb,
                        ):
                            ybufT = ybh.tile([128, 32, 512], bf)
                            for mloc in range(4):
                                m = 4 * half + mloc
                                for kk in range(32):
                                    nc.sync.dma_start_transpose(
                                        ybufT[:, kk, mloc * 128:(mloc + 1) * 128],
                                        estore[:, m, kk * 128:(kk + 1) * 128])
                            for gg in range(4):
                                ups = [psb.tile([128, 8, 64], f32, tag=f"u{j}",
                                                name=f"ups{j}")
                                       for j in range(4)]
                                for k in range(32):
                                    rt = wkc.tile([128, 4, 128], bf, tag="rt")
                                    for j in range(4):
                                        g = 4 * gg + j
                                        di, dj = divmod(g, 4)
                                        nc.sync.dma_start(
                                            rt[:, j],
                                            g_x1[4 * k + di:4 * k + di + 3:2,
                                                 dj:dj + 127:2, :])
                                    for j in range(4):
                                        nc.tensor.matmul(ups[j][:], rt[:, j],
                                                         ybufT[:, k, :],
                                                         start=(k == 0),
                                                         stop=(k == 31))
                                for j in range(4):
                                    g = 4 * gg + j
                                    di, dj = divmod(g, 4)
                                    r0 = di + 1 + 16 * half
                                    sl = window[:, r0:r0 + 15:2, dj:dj + 127:2]
                                    nc.vector.tensor_add(sl, sl, ups[j][:])
                    nc.gpsimd.dma_start(b_w[:], window[:])

            # ---- gather windows, assemble y, final dilated convs ----
            nc.gpsimd.collective_compute(AG, BYP, replica_groups=GROUPS,
                                         ins=[b_w[:]], outs=[g_w[:]])
            with (
                tc.tile_pool(name="convp", bufs=1) as convp,
                tc.tile_pool(name="wkd", bufs=2) as wkd,
                tc.tile_pool(name="psc", bufs=2,
                             space=bass.MemorySpace.PSUM) as psc,
            ):
                y_bf = convp.tile([128, 144, 144], bf)
                fw_sb = convp.tile([128, NSH, 16], bf)
                fb_sb = convp.tile([16, 1], f32)
                for si in range(NSH):
                    nc.sync.dma_start(fw_sb[:, si, :], d_fw[si])
                nc.sync.dma_start(fb_sb[:], p_f32[L:L + 16])
                with tc.tile_pool(name="ypool", bufs=1) as ypool:
                    y_buf = ypool.tile([128, 144, 144], f32)
                    nc.vector.memset(y_buf[:], 0.0)
                    for k in range(4):
                        wstg = wkd.tile([128, 36, 130], f32, tag="wstg")
                        nc.gpsimd.dma_start(wstg[:], g_w[k])
                        t0 = 2 if k == 0 else 1
                        t1 = 34 if k == 3 else 35
                        dst = y_buf[:, 32 * k + 6 + t0:32 * k + 6 + t1, 8:136]
                        nc.vector.tensor_add(dst, dst, wstg[:, t0:t1, 1:129])
                    nc.vector.tensor_copy(y_bf[:], y_buf[:])
                for blk in range(32):             # out row blocks of 4
                    ops = psc.tile([16, 4, 128], f32, tag="ops")
                    for si, (dh, dv) in enumerate(SHIFTS):
                        r0 = 8 + dh + 4 * blk
                        nc.tensor.matmul(ops[:], fw_sb[:, si, :],
                                         y_bf[:, r0:r0 + 4, 8 + dv:8 + dv + 128],
                                         start=(si == 0), stop=(si == NSH - 1))
                    ob = wkd.tile([16, 4, 128], bf, tag="ob")
                    nc.scalar.activation(ob[:], ops[:], Relu, bias=fb_sb[:],
                                         scale=1.0)
                    nc.sync.dma_start(outp[:, 4 * blk:4 * blk + 4, :], ob[:])
    nc.compile()
    return nc


def _get_nc():
    global _NC
    if _NC is None:
        _NC = _build_nc()
        # The custom-call lowering re-serializes the (immutable, already
        # compiled) BIR module on every dispatch (~60ms); serve it cached.
        blob = _NC.to_json_bytes()
        _NC.to_json_bytes = lambda: blob
    return _NC


def _prep_sample(x1s, x2s, masks):
    """Host prep for one sample: shards + vectors (all cheap)."""
    x1tp = np.pad(x1s * 0.25, ((0, 0), (1, 1), (1, 1))).transpose(1, 2, 0)
    x1fl = np.ascontiguousarray(x1tp).astype(BF16).reshape(-1)
    x2p = np.pad(x2s, ((0, 0), (1, 1), (1, 1))).astype(BF16)

    sq = np.pad((x2s * x2s).sum(0), 1)
    n2 = np.zeros((H2, H2), np.float32)
    mp = np.pad(masks, 1)
    ps = np.zeros((H2, H2), np.float32)
    for u in range(3):
        for v in range(3):
            n2 += sq[u:u + H2, v:v + H2]
            ps += mp[u:u + H2, v:v + H2]
    norm = np.sqrt(n2).reshape(-1)
    mm = (ps.reshape(-1) == 0.0).astype(np.float32)
    scalev = (SCALE * mm / np.maximum(norm, EPS_NORM)).astype(np.float32)
    return x1fl, x2p, scalev, mm


def kernel(x1, x2, mask, fw0, fb0, fw1, fb1, fw2, fb2, fw3, fb3):
    from concourse.bass_utils import run_bass_kernel_spmd

    x1 = np.asarray(x1, np.float32)
    x2 = np.asarray(x2, np.float32)
    mask = np.asarray(mask, np.float32)
    fws = [np.asarray(f, np.float32) for f in (fw0, fw1, fw2, fw3)]
    fbs = [np.asarray(f, np.float32) for f in (fb0, fb1, fb2, fb3)]

    fwt = []
    for q, r in enumerate(RATES):
        t = np.zeros((NSH, 128, 16), np.float32)
        for si, (dh, dv) in enumerate(SHIFTS):
            if dh in (-r, 0, r) and dv in (-r, 0, r):
                u, v = dh // r + 1, dv // r + 1
                t[si] = fws[q][:, :, u, v].T
        fwt.append(t.astype(BF16))

    nc = _get_nc()
    in_maps = []
    for s in range(B):
        x1fl, x2p, scalev, mm = _prep_sample(x1[s], x2[s], mask[s, 0])
        mmbf = mm.astype(BF16)
        for q in range(4):
            blob = np.concatenate([
                x1fl[q * X1CH:(q + 1) * X1CH],
                np.ascontiguousarray(x2p[:, 16 * q:16 * q + 18, :]).reshape(-1),
                mmbf,
                fwt[q].reshape(-1),
            ])
            in_maps.append({
                "p_bf": blob,
                "p_f32": np.concatenate([scalev, fbs[q]]),
            })

    res = run_bass_kernel_spmd(nc, in_maps, core_ids=list(range(8)))

    out = np.empty((B, 64, H1, H1), np.float32)
    for s in range(B):
        for q in range(4):
            out[s, 16 * q:16 * (q + 1)] = res.results[4 * s + q]["outp"].astype(np.float32)
    return out
